# revision 1
# baseline (speedup 1.0000x reference)
"""GCN (2-layer + linear + global mean pool) as a distributed Bass kernel
on 8 Trainium2 NeuronCores.

Strategy (dst-sharded message passing):
- Nodes are sharded by row across the 8 cores (12500 each); every edge is
  assigned to the core that owns its destination node.
- GCN normalization is factored as D^-1/2 (A+I) D^-1/2 X = post-scale *
  (segment-sum of pre-scaled rows): dis[src] is folded into the one-hot
  mask, dis[dst] into the PSUM drain.
- Per-edge aggregation is a sequence of one-hot-mask matmuls: for each tile
  of 128 edges targeting one 128-node destination block, a mask
  S[e, d] = dis_src[e] * (dst_e == d) is built on the vector engine
  (iota + is_equal + scale in one tensor_scalar op) and
  S.T @ messages accumulates into the block's PSUM region.
- Layer-1 messages (x[src], 32 wide) are materialized host-side into the
  per-core edge stream (input marshaling); layer-2 messages (h1~[src], 128
  wide) are gathered on-device with dma_gather (4 SWDGE queues,
  <=1024 indices per call, int16 indices via 4 src-range buckets) from an
  AllGather'ed node table.
- Per-graph mean pooling = one more mask-matmul chain + a tiny AllReduce.
"""

import numpy as np
import ml_dtypes

N_NODES = 100000
N_EDGES = 1000000
F_IN = 32
H = 64
H2 = 128
N_GRAPHS = 64
NCORES = 8

bf16 = ml_dtypes.bfloat16


class CFG:
    def __init__(self, n_nodes, n_graphs, bucket, group):
        self.N = n_nodes
        self.NG = n_graphs
        self.V = n_nodes // NCORES
        self.NB = (self.V + 127) // 128
        self.VLAST = self.V - (self.NB - 1) * 128  # rows in last block
        self.BUCKET = bucket                        # src bucket width (<=32768)
        self.NBUCK = (n_nodes + bucket - 1) // bucket
        self.BUCKET_BASES = [k * bucket for k in range(self.NBUCK)]
        self.GROUP = group                          # dst blocks per group
        self.NGROUPS = (self.NB + group - 1) // group

    def groups(self):
        for g in range(self.NGROUPS):
            yield g, list(range(g * self.GROUP, min((g + 1) * self.GROUP, self.NB)))


FULL = CFG(N_NODES, N_GRAPHS, 32768, 4)


def _prep(cfg, x, edge_index, batch, W1, b1, W2, b2, Wrt, brt):
    """Host-side sharding: per-core padded edge streams + local node data."""
    N, V, NB = cfg.N, cfg.V, cfg.NB
    src = np.asarray(edge_index[0], dtype=np.int64)
    dst = np.asarray(edge_index[1], dtype=np.int64)
    batch = np.asarray(batch, dtype=np.int64)
    deg = (np.bincount(dst, minlength=N) + 1.0).astype(np.float32)

    core = dst // V
    ed = dst - core * V
    blk = ed // 128
    buck = np.minimum(src // cfg.BUCKET, cfg.NBUCK - 1)

    counts = np.zeros((NCORES, NB, cfg.NBUCK), np.int64)
    np.add.at(counts, (core, blk, buck), 1)
    # compile-time tile structure: max over cores, 128-edge quantum
    tiles = np.ceil(counts / 128.0).astype(np.int64).max(axis=0)  # [NB, NBUCK]

    # stream order: for g, for k, for b in g  -> per-(g,k,b) segment
    seg_tile_off = np.zeros((NB, cfg.NBUCK), np.int64)
    t = 0
    gk_runs = []  # (g, k, tile_off, n_tiles) in stream order
    for g, blocks in cfg.groups():
        for k in range(cfg.NBUCK):
            run_off = t
            for b in blocks:
                seg_tile_off[b, k] = t
                t += tiles[b, k]
            gk_runs.append((g, k, run_off, t - run_off))
    T = int(t)

    # per-edge slot assignment (vectorized)
    order = np.lexsort((buck + cfg.NBUCK * blk, core))
    key = (core * NB + blk) * cfg.NBUCK + buck
    ksort = key[order]
    starts = np.r_[0, np.flatnonzero(np.diff(ksort)) + 1]
    grp_id = np.zeros(len(ksort), np.int64)
    grp_id[starts[1:]] = 1
    grp_id = np.cumsum(grp_id)
    rank = np.arange(len(ksort)) - starts[grp_id]

    in_maps = []
    x32 = np.asarray(x, np.float32)
    xb = x32.astype(bf16)
    bases = np.array(cfg.BUCKET_BASES)
    for c in range(NCORES):
        sel = core[order] == c
        es = src[order][sel]
        eb = blk[order][sel]
        ek = buck[order][sel]
        er = rank[sel]
        slot = (seg_tile_off[eb, ek] * 128 + er).astype(np.int64)

        eidx = np.zeros(T * 128, np.int16)
        dstq = np.full(T * 128, -1.0, np.float32)
        dsrc = np.ones(T * 128, np.float32)
        xe = np.zeros((T * 128, F_IN), bf16)
        eidx[slot] = (es - bases[ek]).astype(np.int16)
        dstq[slot] = (dst[order][sel] - c * V) % 128
        dsrc[slot] = deg[es]
        xe[slot] = xb[es]

        def pt(a):
            return np.ascontiguousarray(a.reshape(T, 128).T)

        eidx_w = np.tile(np.ascontiguousarray(eidx.reshape(T * 8, 16).T), (8, 1))

        xloc = np.zeros((128, NB, F_IN), np.float32)
        dloc = np.ones((128, NB), np.float32)
        batq = np.full((128, NB), -1.0, np.float32)
        nodes = np.arange(c * V, (c + 1) * V)
        p_i = (nodes - c * V) % 128
        b_i = (nodes - c * V) // 128
        xloc[p_i, b_i] = x32[nodes]
        dloc[p_i, b_i] = deg[nodes]
        batq[p_i, b_i] = batch[nodes]

        in_maps.append({
            "xe": np.ascontiguousarray(xe.reshape(T, 128, F_IN).transpose(1, 0, 2)),
            "eidx": np.ascontiguousarray(eidx_w),
            "dstq": pt(dstq),
            "dsrc": pt(dsrc),
            "xloc": xloc,
            "dloc": dloc,
            "batq": batq,
            "w1": np.asarray(W1, np.float32).astype(bf16),
            "w2": np.asarray(W2, np.float32).astype(bf16),
            "wrt": np.asarray(Wrt, np.float32).astype(bf16),
            "b1": np.asarray(b1, np.float32).reshape(1, -1).astype(bf16),
            "b2": np.asarray(b2, np.float32).reshape(-1, 1).astype(bf16),
            "brt": np.asarray(brt, np.float32).reshape(1, -1),
        })
    return in_maps, tiles, seg_tile_off, gk_runs, T


DEBUG = False


def _build(cfg, tiles, seg_tile_off, gk_runs, T):
    import concourse.mybir as mybir
    import concourse.tile as tile
    from concourse.bacc import Bacc
    from concourse.masks import make_identity

    f32 = mybir.dt.float32
    b16 = mybir.dt.bfloat16
    i16 = mybir.dt.int16
    eq = mybir.AluOpType.is_equal
    mult = mybir.AluOpType.mult
    add = mybir.AluOpType.add
    NB, V, NG = cfg.NB, cfg.V, cfg.NG
    VLAST = cfg.VLAST
    max_gnt = max(
        sum(int(tiles[b, k]) for b in blocks for k in range(cfg.NBUCK))
        for _, blocks in cfg.groups())

    nc = Bacc(trn_type="TRN2", num_devices=NCORES, num_swdge_queues=4)

    t_xe = nc.dram_tensor("xe", [128, T, F_IN], b16, kind="ExternalInput")
    t_eidx = nc.dram_tensor("eidx", [128, T * 8], i16, kind="ExternalInput")
    t_dstq = nc.dram_tensor("dstq", [128, T], f32, kind="ExternalInput")
    t_dsrc = nc.dram_tensor("dsrc", [128, T], f32, kind="ExternalInput")
    t_xloc = nc.dram_tensor("xloc", [128, NB, F_IN], f32, kind="ExternalInput")
    t_dloc = nc.dram_tensor("dloc", [128, NB], f32, kind="ExternalInput")
    t_batq = nc.dram_tensor("batq", [128, NB], f32, kind="ExternalInput")
    t_w1 = nc.dram_tensor("w1", [F_IN, H2], b16, kind="ExternalInput")
    t_w2 = nc.dram_tensor("w2", [H2, H], b16, kind="ExternalInput")
    t_wrt = nc.dram_tensor("wrt", [H, H], b16, kind="ExternalInput")
    t_b1 = nc.dram_tensor("b1", [1, H2], b16, kind="ExternalInput")
    t_b2 = nc.dram_tensor("b2", [H, 1], b16, kind="ExternalInput")
    t_brt = nc.dram_tensor("brt", [1, H], f32, kind="ExternalInput")
    t_out = nc.dram_tensor("out", [NG, NG], f32, kind="ExternalOutput")
    if DEBUG:
        t_dbgz = nc.dram_tensor("dbg_z", [128, NB, F_IN], f32, kind="ExternalOutput")
        t_dbgh1 = nc.dram_tensor("dbg_h1", [128, NB, H2], f32, kind="ExternalOutput")
        t_dbgw = nc.dram_tensor("dbg_w", [128, NB, H2], f32, kind="ExternalOutput")
        t_dbgh3 = nc.dram_tensor("dbg_h3", [128, NB, H + 1], f32, kind="ExternalOutput")

    cc_in = nc.dram_tensor("cc_in", [V, H2], b16, kind="Internal")
    ht = nc.dram_tensor("ht", [cfg.N, H2], b16, kind="Internal", addr_space="Shared")
    ar_in = nc.dram_tensor("ar_in", [NG, NG + 1], f32, kind="Internal")
    ar_out = nc.dram_tensor("ar_out", [NG, NG + 1], f32, kind="Internal", addr_space="Shared")
    rg = [list(range(NCORES))]

    with tile.TileContext(nc) as tc:
        with (
            tc.tile_pool(name="consts", bufs=1) as cp,
            tc.tile_pool(name="sb", bufs=2) as sb,
            tc.tile_pool(name="mask", bufs=4) as mp,
            tc.tile_pool(name="slab", bufs=2) as slp,
            tc.tile_pool(name="psum", bufs=1, space="PSUM") as pp,
        ):
            # ---------- constants ----------
            iota_i = cp.tile([128, 128], mybir.dt.int32)
            nc.gpsimd.iota(iota_i[:], pattern=[[1, 128]], base=0, channel_multiplier=0)
            iota_bf = cp.tile([128, 128], b16)
            nc.vector.tensor_copy(iota_bf[:], iota_i[:])
            iota_g = cp.tile([128, NG], b16)
            nc.vector.tensor_copy(iota_g[:], iota_i[:, :NG])
            iota4_i = cp.tile([128, 4, 128], mybir.dt.int32)
            nc.gpsimd.iota(iota4_i[:], pattern=[[0, 4], [1, 128]], base=0,
                           channel_multiplier=0)
            iota4_bf = cp.tile([128, 4, 128], b16)
            nc.vector.tensor_copy(iota4_bf[:], iota4_i[:])
            ident = cp.tile([128, 128], b16)
            make_identity(nc, ident[:])

            w1_sb = cp.tile([F_IN + 1, H2], b16)       # [W1; b1] augmented
            nc.sync.dma_start(out=w1_sb[:F_IN, :], in_=t_w1[:])
            nc.sync.dma_start(out=w1_sb[F_IN : F_IN + 1, :], in_=t_b1[:])
            w2_sb = cp.tile([H2, H], b16)
            nc.sync.dma_start(out=w2_sb[:], in_=t_w2[:])
            wrt_sb = cp.tile([H + 1, H], b16)          # [Wrt; b2@Wrt + brt]
            nc.sync.dma_start(out=wrt_sb[:H, :], in_=t_wrt[:])
            b2_sb = cp.tile([H, 1], b16)
            nc.sync.dma_start(out=b2_sb[:], in_=t_b2[:])
            brt_sb = cp.tile([1, H], f32)
            nc.sync.dma_start(out=brt_sb[:], in_=t_brt[:])
            brt2_ps = pp.tile([1, H], f32, space="PSUM", tag="dense", bufs=2)
            nc.tensor.matmul(brt2_ps[:], lhsT=b2_sb[:], rhs=wrt_sb[:H, :],
                             start=True, stop=True, skip_group_check=True)
            nc.vector.tensor_add(out=wrt_sb[H : H + 1, :], in0=brt2_ps[:], in1=brt_sb[:])

            # ---------- per-node / per-edge scalar prep ----------
            dstq_sb = cp.tile([128, T], f32)
            nc.sync.dma_start(out=dstq_sb[:], in_=t_dstq[:])
            dis_src = cp.tile([128, T], f32)
            nc.sync.dma_start(out=dis_src[:], in_=t_dsrc[:])
            nc.scalar.sqrt(dis_src[:], dis_src[:])
            nc.vector.reciprocal(dis_src[:], dis_src[:])

            dloc_sb = cp.tile([128, NB], f32)
            nc.sync.dma_start(out=dloc_sb[:], in_=t_dloc[:])
            dis_loc = cp.tile([128, NB], f32)
            nc.scalar.sqrt(dis_loc[:], dloc_sb[:])
            nc.vector.reciprocal(dis_loc[:], dis_loc[:])
            rdeg = cp.tile([128, NB], f32)
            nc.vector.reciprocal(rdeg[:], dloc_sb[:])

            batq_sb = cp.tile([128, NB], f32)
            nc.sync.dma_start(out=batq_sb[:], in_=t_batq[:])

            # xdd = x / deg  (= x * dis^2), the L1 self-loop term pre-scaled
            xdd = cp.tile([128, NB, F_IN], f32)
            xl_t = sb.tile([128, NB, F_IN], f32, tag="xl", bufs=1)
            nc.sync.dma_start(out=xl_t[:], in_=t_xloc[:])
            for b in range(NB):
                nc.vector.tensor_scalar(
                    out=xdd[:, b, :], in0=xl_t[:, b, :],
                    scalar1=rdeg[:, b : b + 1], scalar2=None, op0=mult)

            # augmented-transpose staging tiles with a fixed ones-row
            zt_tiles = [cp.tile([F_IN + 1, 128], b16, tag=f"zt{i}", name=f"zt{i}") for i in range(2)]
            h2t_tiles = [cp.tile([H + 1, 128], b16, tag=f"h2t{i}", name=f"h2t{i}") for i in range(2)]
            for tl in zt_tiles:
                nc.vector.memset(tl[F_IN : F_IN + 1, :], 1.0)
            for tl in h2t_tiles:
                nc.vector.memset(tl[H : H + 1, :], 1.0)

            hdd = cp.tile([128, NB, H2], b16)     # dis^2 * relu(conv1), L2 self term
            h3b = cp.tile([128, NB, H + 1], b16)  # final per-node features + ones col
            nc.vector.memset(h3b[:], 1.0)

            def mask_tile(s, scaled=True):
                S = mp.tile([128, 128], b16, tag="S", bufs=4, name=f"S{s}")
                if scaled:
                    nc.vector.tensor_scalar(
                        out=S[:], in0=iota_bf[:],
                        scalar1=dstq_sb[:, s : s + 1], scalar2=dis_src[:, s : s + 1],
                        op0=eq, op1=mult)
                else:
                    nc.vector.tensor_scalar(
                        out=S[:], in0=iota_bf[:],
                        scalar1=dstq_sb[:, s : s + 1], scalar2=None, op0=eq)
                return S

            import concourse.bass as bass_mod

            def mask4(s0, m):
                S4 = mp.tile([128, 4, 128], b16, tag="S4", bufs=4, name=f"S4_{s0}")
                a = dstq_sb[:, s0 : s0 + m]
                bc = bass_mod.AP(a.tensor, a.offset,
                                 [list(p) for p in a.ap] + [[0, 128]])
                nc.vector.tensor_tensor(out=S4[:, :m, :], in0=bc,
                                        in1=iota4_bf[:, :m, :], op=eq)
                return S4

            def seg_info(blocks):
                first = {b: True for b in blocks}
                last_seg = {}
                for k in range(cfg.NBUCK):
                    for b in blocks:
                        if tiles[b, k]:
                            last_seg[b] = k
                return first, last_seg

            # ================= Layer 1 sweep =================
            for g, blocks in cfg.groups():
                g_t0 = int(seg_tile_off[blocks[0], 0])
                g_nt = sum(int(tiles[b, k]) for b in blocks for k in range(cfg.NBUCK))
                first, last_seg = seg_info(blocks)
                if g_nt:
                    slabx = slp.tile([128, max_gnt, F_IN], b16, tag="slabx", bufs=2)
                    nc.sync.dma_start(out=slabx[:, :g_nt, :],
                                      in_=t_xe[:, g_t0 : g_t0 + g_nt, :])
                    q1b = {}
                    for b in blocks:
                        q1b[b] = pp.tile([128, F_IN], f32, space="PSUM",
                                         tag="qacc", bufs=cfg.GROUP,
                                         name=f"q1_{g}_{b}")
                    for b in blocks:
                        for k in range(cfg.NBUCK):
                            nt = int(tiles[b, k])
                            if nt == 0:
                                continue
                            t0 = int(seg_tile_off[b, k])
                            for i in range(nt):
                                s = t0 + i
                                S = mask_tile(s)
                                nc.tensor.matmul(
                                    q1b[b][:], lhsT=S[:],
                                    rhs=slabx[:, s - g_t0, :],
                                    start=first[b],
                                    stop=(k == last_seg[b] and i == nt - 1),
                                    skip_group_check=True)
                                first[b] = False
                # drain + dense chain per block
                for b in blocks:
                    z = sb.tile([128, F_IN], b16, tag="z")
                    if b in last_seg:
                        nc.vector.scalar_tensor_tensor(
                            out=z[:], in0=q1b[b][:],
                            scalar=dis_loc[:, b : b + 1], in1=xdd[:, b, :],
                            op0=mult, op1=add)
                    else:
                        nc.vector.tensor_copy(z[:], xdd[:, b, :])
                    zt_ps = pp.tile([F_IN, 128], b16, space="PSUM", tag="dense", bufs=2)
                    nc.tensor.transpose(out=zt_ps[:], in_=z[:], identity=ident[:])
                    zt = zt_tiles[b % 2]
                    nc.scalar.copy(zt[:F_IN, :], zt_ps[:])
                    h1_ps = pp.tile([128, H2], f32, space="PSUM", tag="dense", bufs=2)
                    nc.tensor.matmul(h1_ps[:], lhsT=zt[:], rhs=w1_sb[:],
                                     start=True, stop=True, skip_group_check=True)
                    if DEBUG:
                        zf = sb.tile([128, F_IN], f32, tag="zf")
                        nc.vector.tensor_copy(zf[:], z[:])
                        nc.sync.dma_start(out=t_dbgz[:, b, :], in_=zf[:])
                    h1t = sb.tile([128, H2], b16, tag="h1t")
                    nc.scalar.activation(h1t[:], h1_ps[:],
                                         mybir.ActivationFunctionType.Relu)
                    if DEBUG:
                        h1f = sb.tile([128, H2], f32, tag="h1f")
                        nc.vector.tensor_copy(h1f[:], h1t[:])
                        nc.sync.dma_start(out=t_dbgh1[:, b, :], in_=h1f[:])
                    htl = sb.tile([128, H2], b16, tag="htl")
                    nc.vector.tensor_scalar(
                        out=htl[:], in0=h1t[:],
                        scalar1=dis_loc[:, b : b + 1], scalar2=None, op0=mult)
                    nc.vector.tensor_scalar(
                        out=hdd[:, b, :], in0=htl[:],
                        scalar1=dis_loc[:, b : b + 1], scalar2=None, op0=mult)
                    if b < NB - 1:
                        nc.sync.dma_start(out=cc_in[b * 128 : (b + 1) * 128, :],
                                          in_=htl[:])
                    else:
                        nc.sync.dma_start(out=cc_in[b * 128 :, :], in_=htl[:VLAST, :])

            # ---------- exchange h1~ ----------
            nc.gpsimd.collective_compute(
                "AllGather", mybir.AluOpType.bypass,
                ins=[cc_in[:]], outs=[ht[:]], replica_groups=rg)

            # ================= Layer 2 sweep =================
            call_i = 0
            pool_ps = pp.tile([NG, NG + 1], f32, space="PSUM", tag="pool", bufs=1)
            for g, blocks in cfg.groups():
                g_t0 = int(seg_tile_off[blocks[0], 0])
                g_nt = sum(int(tiles[b, k]) for b in blocks for k in range(cfg.NBUCK))
                first, last_seg = seg_info(blocks)
                slab_of = {}
                if g_nt:
                    idx_g = slp.tile([128, max_gnt * 8], i16, tag="idxg", bufs=2)
                    nc.sync.dma_start(out=idx_g[:, : g_nt * 8],
                                      in_=t_eidx[:, g_t0 * 8 : (g_t0 + g_nt) * 8])
                    for gg, k, run_off, run_nt in gk_runs:
                        if gg != g or run_nt == 0:
                            continue
                        base = cfg.BUCKET_BASES[k]
                        rows = min(cfg.BUCKET, cfg.N - base)
                        pos = 0
                        while pos < run_nt:
                            nt = min(8, run_nt - pos)          # <=1024 idxs per call
                            toff = run_off + pos
                            nidx = nt * 128
                            o = toff - g_t0
                            cslab = slp.tile([128, 8, H2], b16, tag="slab2", bufs=8,
                                             name=f"cslab{call_i}")
                            for j in range(nt):
                                slab_of[toff + j] = (cslab, j)
                            nc.gpsimd.dma_gather(
                                out_ap=cslab[:, :nt, :],
                                in_ap=ht[base : base + rows, :],
                                idxs_ap=idx_g[:, o * 8 : o * 8 + nidx // 16],
                                num_idxs=nidx, num_idxs_reg=nidx, elem_size=H2,
                                queue_num=call_i % 4)
                            call_i += 1
                            pos += nt
                    q2b_ = {}
                    for b in blocks:
                        q2b_[b] = pp.tile([128, H2], f32, space="PSUM",
                                          tag="qacc", bufs=cfg.GROUP,
                                          name=f"q2_{g}_{b}")

                    def q2(j, _d=q2b_, _b0=blocks[0]):
                        return _d[_b0 + j][:]

                    for b in blocks:
                        for k in range(cfg.NBUCK):
                            nt = int(tiles[b, k])
                            if nt == 0:
                                continue
                            t0 = int(seg_tile_off[b, k])
                            for i0 in range(0, nt, 4):
                                m = min(4, nt - i0)
                                S4 = mask4(t0 + i0, m)
                                for j in range(m):
                                    i = i0 + j
                                    s = t0 + i
                                    ctile, cj = slab_of[s]
                                    nc.tensor.matmul(
                                        q2(b - blocks[0]), lhsT=S4[:, j, :],
                                        rhs=ctile[:, cj, :],
                                        start=first[b],
                                        stop=(k == last_seg[b] and i == nt - 1),
                                        skip_group_check=True)
                                    first[b] = False
                for b in blocks:
                    w = sb.tile([128, H2], b16, tag="w")
                    if b in last_seg:
                        nc.vector.scalar_tensor_tensor(
                            out=w[:], in0=q2(b - blocks[0]),
                            scalar=dis_loc[:, b : b + 1], in1=hdd[:, b, :],
                            op0=mult, op1=add)
                    else:
                        nc.vector.tensor_copy(w[:], hdd[:, b, :])
                    if DEBUG:
                        wf = sb.tile([128, H2], f32, tag="wf")
                        nc.vector.tensor_copy(wf[:], w[:])
                        nc.sync.dma_start(out=t_dbgw[:, b, :], in_=wf[:])
                    wt_ps = pp.tile([H2, 128], b16, space="PSUM", tag="dense", bufs=2)
                    nc.tensor.transpose(out=wt_ps[:], in_=w[:], identity=ident[:])
                    wt = sb.tile([H2, 128], b16, tag="wt")
                    nc.scalar.copy(wt[:], wt_ps[:])
                    h2_ps = pp.tile([128, H], f32, space="PSUM", tag="dense", bufs=2)
                    nc.tensor.matmul(h2_ps[:], lhsT=wt[:], rhs=w2_sb[:],
                                     start=True, stop=True, skip_group_check=True)
                    h2 = sb.tile([128, H], b16, tag="h2")
                    nc.scalar.copy(h2[:], h2_ps[:])
                    h2t_ps = pp.tile([H, 128], b16, space="PSUM", tag="dense", bufs=2)
                    nc.tensor.transpose(out=h2t_ps[:], in_=h2[:], identity=ident[:])
                    h2t = h2t_tiles[b % 2]
                    nc.scalar.copy(h2t[:H, :], h2t_ps[:])
                    h3_ps = pp.tile([128, H], f32, space="PSUM", tag="dense", bufs=2)
                    nc.tensor.matmul(h3_ps[:], lhsT=h2t[:], rhs=wrt_sb[:],
                                     start=True, stop=True, skip_group_check=True)
                    nc.scalar.copy(h3b[:, b, :H], h3_ps[:])
                    # pooled accumulation
                    G = mp.tile([128, NG], b16, tag="G", bufs=4)
                    nc.vector.tensor_scalar(
                        out=G[:], in0=iota_g[:],
                        scalar1=batq_sb[:, b : b + 1], scalar2=None, op0=eq)
                    nc.tensor.matmul(
                        pool_ps[:], lhsT=G[:], rhs=h3b[:, b, :],
                        start=(b == 0), stop=(b == NB - 1), skip_group_check=True)

            if DEBUG:
                h3f = sb.tile([128, NB, H + 1], f32, tag="h3f", bufs=1)
                nc.vector.tensor_copy(h3f[:], h3b[:])
                nc.sync.dma_start(out=t_dbgh3[:], in_=h3f[:])
            pooled = sb.tile([NG, NG + 1], f32, tag="pooled")
            nc.vector.tensor_copy(pooled[:], pool_ps[:])
            nc.sync.dma_start(out=ar_in[:], in_=pooled[:])
            nc.gpsimd.collective_compute(
                "AllReduce", mybir.AluOpType.add,
                ins=[ar_in[:]], outs=[ar_out[:]], replica_groups=rg)
            fin = sb.tile([NG, NG + 1], f32, tag="fin")
            nc.sync.dma_start(out=fin[:], in_=ar_out[:])
            cnt = sb.tile([NG, 1], f32, tag="cnt")
            nc.vector.tensor_scalar_max(cnt[:], fin[:, NG : NG + 1], 1.0)
            rcnt = sb.tile([NG, 1], f32, tag="rcnt")
            nc.vector.reciprocal(rcnt[:], cnt[:])
            outv = sb.tile([NG, NG], f32, tag="outv")
            nc.vector.tensor_scalar(
                out=outv[:], in0=fin[:, :NG], scalar1=rcnt[:], scalar2=None, op0=mult)
            nc.sync.dma_start(out=t_out[:], in_=outv[:])

    nc.compile()
    return nc


def _kernel(cfg, inputs, use_sim=False):
    in_maps, tiles, seg_tile_off, gk_runs, T = _prep(cfg, **inputs)
    nc = _build(cfg, tiles, seg_tile_off, gk_runs, T)
    if use_sim:
        import concourse.bass_interp as bass_interp
        sim = bass_interp.MultiCoreSim(nc, NCORES)
        for c in range(NCORES):
            for kk, vv in in_maps[c].items():
                sim.cores[c].tensor(kk)[:] = vv
        sim.simulate(check_with_hw=False)
        return np.asarray(sim.cores[0].mem_tensor("out")).copy(), None
    from concourse.bass_utils import run_bass_kernel_spmd
    res = run_bass_kernel_spmd(nc, in_maps, core_ids=list(range(NCORES)))
    return np.asarray(res.results[0]["out"]), res


def kernel(x, edge_index, batch, W1, b1, W2, b2, Wrt, brt):
    out, _ = _kernel(FULL, dict(x=x, edge_index=edge_index, batch=batch,
                                W1=W1, b1=b1, W2=W2, b2=b2, Wrt=Wrt, brt=brt))
    return out

